# revision 14
# baseline (speedup 1.0000x reference)
"""GAT message-passing kernel for Trainium2, 8 NeuronCores.

Math (per head i, 3 sequential heads):
    h_i  = h @ W_i.T / sqrt(N)
    att  = exp(h_i @ h.T) * adj ; att /= rowsum(att)
    h    = att @ h ; h_out = concat(h_out, h)
logits = h_out @ W_out.T

Device strategy: shard query rows (m) across 8 cores. Everything on-chip is
kept in "transposed" layout attT[k, m] so that both big matmuls are natural:
  scores: attT[k_tile, m] = hT[:, k_tile].T @ h_iT[:, m]        (K = F = 3)
  AV:     av[f, m]       += h[k_tile, f].T @ attT[k_tile, m]    (K = 128)
A parallel ones-stationary matmul accumulates the softmax denominator.
adj is pre-transposed per core on the host, cast to bf16, and stays
resident in SBUF across all 3 iterations (read from HBM exactly once).
h is exchanged between iterations with a tiny AllGather (12 KB).
All engine APs start at partition 0/32/64/96 (hardware constraint).
"""

import numpy as np
import ml_dtypes

N = 8192
F = 3
H = 4
C = 8
NCORES = 8
LOOPS = H - 1
SQRT_N = float(np.sqrt(np.float32(N)))

_CACHE = {}
LAST_RESULT = None  # BassKernelResults of the most recent kernel() call


def _build(n, ncores, pack=5, coll=1, castdma=1):
    import concourse.bass as bass
    import concourse.mybir as mybir
    from concourse import bacc
    from concourse.tile import TileContext

    bf = mybir.dt.bfloat16
    f32 = mybir.dt.float32
    mult = mybir.AluOpType.mult

    r = n // ncores          # rows (queries) per core
    kt = n // 128            # number of 128-wide key tiles
    mc = max(r // 512, 1)    # matmul N-chunks over m
    mw = min(r, 512)         # matmul moving width
    loops = LOOPS

    nc = bacc.Bacc(
        "TRN2", target_bir_lowering=False, debug=False, num_devices=ncores
    )

    adjT_d = nc.dram_tensor("adjT", [n, r], bf, kind="ExternalInput")
    xT_d = nc.dram_tensor("xT", [F, n], f32, kind="ExternalInput")
    xTb_d = nc.dram_tensor("xTb", [F, n], bf, kind="ExternalInput")
    xoT_d = nc.dram_tensor("xoT", [F, r], f32, kind="ExternalInput")
    ws_d = nc.dram_tensor("ws", [loops, F, F], bf, kind="ExternalInput")
    wo_d = nc.dram_tensor("wo", [F, (loops + 1) * C], f32, kind="ExternalInput")
    id_d = nc.dram_tensor("ident", [128, 128], bf, kind="ExternalInput")
    lo_d = nc.dram_tensor("logitsT", [C, r], f32, kind="ExternalOutput")

    psc, ptr, pdn = pack & 1, pack & 2, pack & 4
    ngrp_sc = 4 if psc else 1
    ngrp_tr = 4 if ptr else 1

    with TileContext(nc) as tc:
        with (
            tc.tile_pool(name="persist", bufs=1) as P,
            tc.tile_pool(name="work", bufs=3) as W,
            tc.tile_pool(name="psA", bufs=2, space="PSUM") as PSA,
            tc.tile_pool(name="psB", bufs=2, space="PSUM") as PSB,
            tc.tile_pool(name="dram", bufs=1, space="DRAM") as D,
        ):
            # ---- persistent SBUF state ----
            adj_sb = P.tile([128, kt * r], bf, name="adj_sb")
            hTrep = P.tile([128, n], bf, name="hTrep")     # hT replicas at part 0/32/64/96
            hiTrep = P.tile([128, r], bf, name="hiTrep")   # h_iT replicas at 0/32/64/96
            hNat = P.tile([128, kt * F], bf, name="hNat")  # h natural, 3 cols per k-tile
            ones_sb = P.tile([128, F], bf, name="ones_sb")
            xoT = P.tile([F, r], f32, name="xoT")
            hN = [P.tile([F, r], f32, name=f"hN{i}") for i in range(loops)]
            hTob = P.tile([F, r], bf, name="hTob")         # own hT (bf16), per iter
            ident = P.tile([128, 128], bf, name="ident")
            ws_sb = P.tile([F, loops * F], bf, name="ws_sb")
            wo_sb = P.tile([F, (loops + 1) * C], f32, name="wo_sb")

            nc.vector.memset(ones_sb[:, :], 1.0)
            nc.sync.dma_start(ident[:, :], id_d[:, :])

            # adj row-block (transposed) -> SBUF, once
            for t in range(kt):
                nc.sync.dma_start(
                    adj_sb[:, t * r:(t + 1) * r], adjT_d[t * 128:(t + 1) * 128, :]
                )

            # weights + x
            for i in range(loops):
                nc.sync.dma_start(ws_sb[:, i * F:(i + 1) * F], ws_d[i])
            nc.sync.dma_start(wo_sb[:, :], wo_d[:, :])
            nc.sync.dma_start(xoT[:, :], xoT_d[:, :])
            for j in range(4):
                if castdma:
                    nc.gpsimd.dma_start(hTrep[32 * j:32 * j + F, :], xT_d[:, :])
                else:
                    nc.sync.dma_start(hTrep[32 * j:32 * j + F, :], xTb_d[:, :])

            for i in range(loops):
                hT_own = xoT if i == 0 else hN[i - 1]

                # ---- hNat: transpose hT into natural layout ----
                tr_ps = PSB.tile([128, kt * 4], bf, name="tr_ps", tag="small")
                for t in range(kt):
                    j = t % ngrp_tr
                    nc.tensor.transpose(
                        tr_ps[:, 4 * t:4 * t + F],
                        hTrep[32 * j:32 * j + F, 128 * t:128 * (t + 1)],
                        ident[32 * j:32 * j + F, 32 * j:32 * j + F],
                        tile_position=(32 * j, 0) if ptr else None,
                    )
                nc.vector.tensor_copy(
                    hNat[:, :].rearrange("p (t q) -> p t q", q=F),
                    tr_ps[:, :].rearrange("p (t q) -> p t q", q=4)[:, :, 0:F],
                )

                # ---- h_iT = (W_i/sqrt(N)) @ hT_own ----
                nc.vector.tensor_copy(hTob[:, :], hT_own[:, :])
                hi_ps = PSB.tile([F, r], f32, name="hi_ps", tag="small")
                for c in range(mc):
                    nc.tensor.matmul(
                        hi_ps[:, c * mw:(c + 1) * mw],
                        ws_sb[:, i * F:(i + 1) * F],
                        hTob[:, c * mw:(c + 1) * mw],
                        start=True, stop=True,
                    )
                nc.vector.tensor_copy(hiTrep[0:F, :], hi_ps[:, :])
                for j in range(1, 4):
                    nc.vector.tensor_copy(hiTrep[32 * j:32 * j + F, :], hiTrep[0:F, :])

                # ---- main stream over key tiles ----
                av_ps = PSB.tile([128, r], f32, name="av_ps", tag="small")
                dn_ps = PSB.tile([128, r], f32, name="dn_ps", tag="small")
                dnp = 64 if pdn else 0
                for t in range(kt):
                    j = t % ngrp_sc  # scores row-group
                    sc_ps = PSA.tile([128, r], f32, name="sc_ps", tag="sc")
                    for c in range(mc):
                        nc.tensor.matmul(
                            sc_ps[:, c * mw:(c + 1) * mw],
                            hTrep[32 * j:32 * j + F, 128 * t:128 * (t + 1)],
                            hiTrep[32 * j:32 * j + F, c * mw:(c + 1) * mw],
                            start=True, stop=True,
                            tile_position=(32 * j, 0) if psc else None,
                        )
                    ex_sb = W.tile([128, r], bf, name="ex_sb", tag="ex")
                    nc.scalar.activation(
                        ex_sb[:, :], sc_ps[:, :], mybir.ActivationFunctionType.Exp
                    )
                    at_sb = W.tile([128, r], bf, name="at_sb", tag="at")
                    nc.vector.tensor_tensor(
                        at_sb[:, :], ex_sb[:, :], adj_sb[:, t * r:(t + 1) * r], op=mult
                    )
                    for c in range(mc):
                        nc.tensor.matmul(
                            av_ps[0:F, c * mw:(c + 1) * mw],
                            hNat[:, F * t:F * (t + 1)],
                            at_sb[:, c * mw:(c + 1) * mw],
                            start=(t == 0), stop=(t == kt - 1),
                            tile_position=(0, 0) if pdn else None,
                        )
                        nc.tensor.matmul(
                            dn_ps[dnp:dnp + F, c * mw:(c + 1) * mw],
                            ones_sb[:, :],
                            at_sb[:, c * mw:(c + 1) * mw],
                            start=(t == 0), stop=(t == kt - 1),
                            tile_position=(0, dnp) if pdn else None,
                        )

                # ---- normalize: hN = av / denom ----
                rc = W.tile([F, r], f32, name="rc", tag="rc")
                nc.vector.reciprocal(rc[:, :], dn_ps[dnp:dnp + F, :])
                nc.vector.tensor_tensor(hN[i][:, :], av_ps[0:F, :], rc[:, :], op=mult)

                # ---- exchange h across cores ----
                if i < loops - 1:
                    if coll:
                        ag_in = D.tile([F, r], f32, name="ag_in", tag=f"agin{i}")
                        ag_out = D.tile(
                            [ncores * F, r], f32, name="ag_out",
                            tag=f"agout{i}", addr_space="Shared",
                        )
                        nc.sync.dma_start(ag_in[:, :], hN[i][:, :])
                        nc.gpsimd.collective_compute(
                            "AllGather",
                            mybir.AluOpType.bypass,
                            replica_groups=[list(range(ncores))],
                            ins=[ag_in[:, :].opt()],
                            outs=[ag_out[:, :].opt()],
                        )
                        src = ag_out[:, :].rearrange("(g f) m -> f g m", f=F)
                        if castdma:
                            for j in range(4):
                                nc.gpsimd.dma_start(
                                    hTrep[32 * j:32 * j + F, :].rearrange(
                                        "f (g m) -> f g m", g=ncores
                                    ),
                                    src,
                                )
                        else:
                            hTf = W.tile([F, n], f32, name="hTf", tag="hTf")
                            nc.sync.dma_start(
                                hTf[:, :].rearrange("f (g m) -> f g m", g=ncores),
                                src,
                            )
                            for j in range(4):
                                nc.vector.tensor_copy(
                                    hTrep[32 * j:32 * j + F, :], hTf[:, :]
                                )
                    else:
                        # no-collective stub: own block only (wrong results)
                        hNb = W.tile([F, r], bf, name="hNb", tag="hNb")
                        nc.vector.tensor_copy(hNb[:, :], hN[i][:, :])
                        for j in range(4):
                            nc.vector.tensor_copy(
                                hTrep[32 * j:32 * j + F, 0:r], hNb[:, :]
                            )

            # ---- logits: accumulate over the 4 concat blocks ----
            lg_ps = PSB.tile([C, r], f32, name="lg_ps", tag="small")
            blocks = [xoT] + hN
            for b in range(loops + 1):
                for c in range(mc):
                    nc.tensor.matmul(
                        lg_ps[:, c * mw:(c + 1) * mw],
                        wo_sb[:, b * C:(b + 1) * C],
                        blocks[b][:, c * mw:(c + 1) * mw],
                        start=(b == 0), stop=(b == loops),
                    )
            lo_sb = W.tile([C, r], f32, name="lo_sb", tag="lo")
            nc.vector.tensor_copy(lo_sb[:, :], lg_ps[:, :])
            nc.sync.dma_start(lo_d[:, :], lo_sb[:, :])

    nc.compile()
    return nc


def prep_inputs(x, adj, W_heads, W_out, n=N, ncores=NCORES):
    """Host-side sharding/preprocessing. Returns per-core input maps."""
    r = n // ncores
    x2 = np.asarray(x, np.float32).reshape(n, F)
    adj2 = np.asarray(adj, np.float32).reshape(n, n)
    xT = np.ascontiguousarray(x2.T)
    sqn = float(np.sqrt(np.float32(n)))
    ws = np.ascontiguousarray(
        np.transpose(np.asarray(W_heads, np.float32)[:LOOPS] / sqn, (0, 2, 1))
    ).astype(ml_dtypes.bfloat16)
    # wo[f, b*C + c] = W_out[c, 3b + f]  (block b of W_out.T)
    woT = np.asarray(W_out, np.float32).T  # [(loops+1)*F, C]
    wo = np.ascontiguousarray(np.concatenate(
        [woT[b * F:(b + 1) * F, :] for b in range(LOOPS + 1)], axis=1
    ))
    ident = np.eye(128, dtype=ml_dtypes.bfloat16)
    in_maps = []
    for c in range(ncores):
        rows = slice(c * r, (c + 1) * r)
        adjT = np.ascontiguousarray(adj2[rows, :].T).astype(ml_dtypes.bfloat16)
        in_maps.append({
            "adjT": adjT,
            "xT": xT,
            "xTb": xT.astype(ml_dtypes.bfloat16),
            "xoT": np.ascontiguousarray(xT[:, rows]),
            "ws": ws,
            "wo": wo,
            "ident": ident,
        })
    return in_maps


def kernel(x, adj, W_heads, W_out):
    from concourse import bass_utils

    key = (N, NCORES)
    if key not in _CACHE:
        _CACHE[key] = _build(N, NCORES)
    nc = _CACHE[key]

    in_maps = prep_inputs(x, adj, W_heads, W_out)
    res = bass_utils.run_bass_kernel_spmd(
        nc, in_maps, core_ids=list(range(NCORES))
    )
    global LAST_RESULT
    LAST_RESULT = res
    r = N // NCORES
    out = np.empty((1, N, C), np.float32)
    for c in range(NCORES):
        out[0, c * r:(c + 1) * r, :] = res.results[c]["logitsT"].T
    return out
